# revision 1
# baseline (speedup 1.0000x reference)
"""Bass/Trainium2 kernel for nn_CMOS_60181081752266.

Computes, for each of 10 classes, sum(|patch|) where patch is a static
273x273 crop of the 8192x8192 input X. Only ~3MB of X is ever needed, so
the host slices the 10 patches out of X and repacks them across
8 cores x 128 SBUF partitions such that every partition row holds data
from exactly one class. Each core then runs a tiny kernel:

    DMA in [128, 731] f32  ->  DVE tensor_reduce(|x|, sum over free axis)
    ->  DMA out [128, 1] per-partition sums

and the host adds up each class's 102 partition sums.
"""

import numpy as np

import concourse.bass as bass
import concourse.mybir as mybir
from concourse.bass_utils import run_bass_kernel_spmd

CLASSES = 10
FRAME_S = 273          # 8192 // (10*3)
GRID = 8192
N_CORES = 8
PPC = 102              # partitions per class (10*102 = 1020 <= 1024)
F = 731                # ceil(273*273 / 102); 102*731 = 74562 >= 74529
P = 128                # SBUF partitions per core


def _starts():
    # cols = floor(sqrt(10)) + 1 = 4, cell = 8192 // 4 = 2048
    # xs = (i%4)*2048 + 1024 - 136, ys = (i//4)*2048 + 1024 - 136
    return [(888 + 2048 * (i % 4), 888 + 2048 * (i // 4)) for i in range(CLASSES)]


def _build_bass():
    nc = bass.Bass()
    x = nc.dram_tensor("x", [P, F], mybir.dt.float32, kind="ExternalInput")
    y = nc.dram_tensor("y", [P, 1], mybir.dt.float32, kind="ExternalOutput")
    with (
        nc.sbuf_tensor("t", [P, F], mybir.dt.float32) as t,
        nc.sbuf_tensor("acc", [P, 1], mybir.dt.float32) as acc,
        nc.semaphore("dsem") as dsem,
        nc.semaphore("vsem") as vsem,
        nc.Block() as block,
    ):

        @block.sync
        def _(sync):
            sync.dma_start(t[:], x[:]).then_inc(dsem, 16)
            sync.wait_ge(vsem, 1)
            sync.dma_start(y[:], acc[:]).then_inc(dsem, 16)
            sync.wait_ge(dsem, 32)

        @block.vector
        def _(vector):
            vector.wait_ge(dsem, 16)
            vector.tensor_reduce(
                acc[:],
                t[:],
                axis=mybir.AxisListType.X,
                op=mybir.AluOpType.add,
                apply_absolute_value=True,
            ).then_inc(vsem, 1)

    return nc


_NC = None


def kernel(X: np.ndarray) -> np.ndarray:
    global _NC
    X = np.ascontiguousarray(X, dtype=np.float32)

    # Pack the 10 patches into [1024, 731]; class c owns rows
    # [102c, 102c+102), rows 1020..1023 are zero padding.
    G = np.zeros((N_CORES * P, F), dtype=np.float32)
    for c, (xs, ys) in enumerate(_starts()):
        flat = X[xs : xs + FRAME_S, ys : ys + FRAME_S].reshape(-1)
        buf = np.zeros(PPC * F, dtype=np.float32)
        buf[: FRAME_S * FRAME_S] = flat
        G[PPC * c : PPC * (c + 1)] = buf.reshape(PPC, F)

    if _NC is None:
        _NC = _build_bass()

    in_maps = [{"x": np.ascontiguousarray(G[P * k : P * (k + 1)])} for k in range(N_CORES)]
    res = run_bass_kernel_spmd(_NC, in_maps, core_ids=list(range(N_CORES)))

    sums = np.concatenate([res.results[k]["y"].reshape(-1) for k in range(N_CORES)])
    out = np.empty(CLASSES, dtype=np.float32)
    for c in range(CLASSES):
        out[c] = sums[PPC * c : PPC * (c + 1)].sum(dtype=np.float32)
    return out


# revision 5
# speedup vs baseline: 1.3784x; 1.3784x over previous
"""Bass/Trainium2 kernel for nn_CMOS_60181081752266.

Computes, for each of 10 classes, sum(|patch|) where patch is a static
273x273 crop of the 8192x8192 input X. Only ~3MB of X is ever needed, so
the host slices the 10 patches out of X and repacks them across
8 cores x 128 SBUF partitions such that every partition row holds data
from exactly one class (102 partitions per class, 731 floats each).

Per core:
  - input x [128, 731] f32 is DMA'd in 4 column chunks, alternating
    between the two HWDGE queues (sync + scalar) to hide HBM latency
  - DVE reduces each chunk with apply_absolute_value -> acc [128, 4]
  - acc -> racc [128, 1]; multiplied by a per-core one-hot class mask
    [128, 32]; 32x32 block transposes + reduce collapse the 102
    partition sums of each class into one value, landing all 10 class
    sums in a single partition row
  - one 40B DMA (single descriptor -> single DRAM-write receipt) writes
    y [1, 10]

Host adds the per-core y vectors (each core only covers 2-3 classes;
the mask zeroes the rest).
"""

import numpy as np

import concourse.bass as bass
import concourse.mybir as mybir
from concourse.bass_utils import run_bass_kernel_spmd

CLASSES = 10
FRAME_S = 273          # 8192 // (10*3)
N_CORES = 8
PPC = 102              # partitions per class (10*102 = 1020 <= 1024)
F = 731                # ceil(273*273 / 102); 102*731 = 74562 >= 74529
P = 128                # SBUF partitions per core
MCOLS = 32             # mask columns (10 used, rest zero)
CUTS = [0, 183, 366, 549, 731]  # column chunk boundaries


def _starts():
    # cols = floor(sqrt(10)) + 1 = 4, cell = 8192 // 4 = 2048
    # xs = (i%4)*2048 + 1024 - 136, ys = (i//4)*2048 + 1024 - 136
    return [(888 + 2048 * (i % 4), 888 + 2048 * (i // 4)) for i in range(CLASSES)]


def _build_bass():
    f32 = mybir.dt.float32
    nc = bass.Bass()
    x = nc.dram_tensor("x", [P, F], f32, kind="ExternalInput")
    mask = nc.dram_tensor("mask", [P, MCOLS], f32, kind="ExternalInput")
    y = nc.dram_tensor("y", [1, CLASSES], f32, kind="ExternalOutput")
    with (
        nc.sbuf_tensor("t", [P, F], f32) as t,
        nc.sbuf_tensor("acc", [P, 4], f32) as acc,
        nc.sbuf_tensor("rtile", [P, MCOLS], f32) as rtile,
        nc.sbuf_tensor("msk", [P, MCOLS], f32) as msk,
        nc.sbuf_tensor("macc", [P, MCOLS], f32) as macc,
        nc.sbuf_tensor("tt", [MCOLS, P], f32) as tt,
        nc.sbuf_tensor("ztile", [MCOLS, MCOLS], f32) as ztile,
        nc.sbuf_tensor("zt2", [MCOLS, MCOLS], f32) as zt2,
        nc.semaphore("c0sem") as c0sem,
        nc.semaphore("c1sem") as c1sem,
        nc.semaphore("c2sem") as c2sem,
        nc.semaphore("c3sem") as c3sem,
        nc.semaphore("ysem") as ysem,
        nc.semaphore("msem") as msem,
        nc.semaphore("gsem") as gsem,
        nc.semaphore("vsem") as vsem,
        nc.Block() as block,
    ):

        @block.gpsimd
        def _(gpsimd):
            gpsimd.memset(rtile[:], 0.0)
            gpsimd.memset(ztile[:], 0.0).then_inc(gsem, 1)
            gpsimd.dma_start(msk[:], mask[:]).then_inc(msem, 16)

        @block.sync
        def _(sync):
            sync.dma_start(t[:, CUTS[0] : CUTS[1]], x[:, CUTS[0] : CUTS[1]]).then_inc(
                c0sem, 16
            )
            sync.dma_start(t[:, CUTS[2] : CUTS[3]], x[:, CUTS[2] : CUTS[3]]).then_inc(
                c2sem, 16
            )
            sync.wait_ge(vsem, 1)
            sync.dma_start(y[:], zt2[0:1, 0:CLASSES]).then_inc(ysem, 16)
            sync.wait_ge(ysem, 16)

        @block.scalar
        def _(scalar):
            scalar.dma_start(
                t[:, CUTS[1] : CUTS[2]], x[:, CUTS[1] : CUTS[2]]
            ).then_inc(c1sem, 16)
            scalar.dma_start(
                t[:, CUTS[3] : CUTS[4]], x[:, CUTS[3] : CUTS[4]]
            ).then_inc(c3sem, 16)

        @block.vector
        def _(vector):
            waits = [(c0sem, 16), (c1sem, 16), (c2sem, 16), (c3sem, 16)]
            for j in range(4):
                sem, val = waits[j]
                vector.wait_ge(sem, val)
                vector.tensor_reduce(
                    acc[:, j : j + 1],
                    t[:, CUTS[j] : CUTS[j + 1]],
                    axis=mybir.AxisListType.X,
                    op=mybir.AluOpType.add,
                    apply_absolute_value=True,
                )
            vector.wait_ge(gsem, 1)
            vector.drain()
            vector.tensor_reduce(
                rtile[:, 0:1],
                acc[:],
                axis=mybir.AxisListType.X,
                op=mybir.AluOpType.add,
            )
            vector.wait_ge(msem, 16)
            vector.drain()
            vector.tensor_scalar(
                macc[:],
                msk[:],
                rtile[:, 0:1],
                None,
                mybir.AluOpType.mult,
            )
            vector.drain()
            for b in range(P // MCOLS):
                vector.transpose(
                    tt[0:MCOLS, b * MCOLS : (b + 1) * MCOLS],
                    macc[b * MCOLS : (b + 1) * MCOLS, 0:MCOLS],
                )
            vector.drain()
            vector.tensor_reduce(
                ztile[:, 0:1],
                tt[:],
                axis=mybir.AxisListType.X,
                op=mybir.AluOpType.add,
            )
            vector.drain()
            vector.transpose(zt2[:], ztile[:]).then_inc(vsem, 1)

    return nc


def _prep_in_maps(X: np.ndarray):
    X = np.ascontiguousarray(X, dtype=np.float32)
    # Pack the 10 patches into [1024, 731]; class c owns global partition
    # rows [102c, 102c+102), rows 1020..1023 are zero padding.
    G = np.zeros((N_CORES * P, F), dtype=np.float32)
    for c, (xs, ys) in enumerate(_starts()):
        flat = X[xs : xs + FRAME_S, ys : ys + FRAME_S].reshape(-1)
        buf = np.zeros(PPC * F, dtype=np.float32)
        buf[: FRAME_S * FRAME_S] = flat
        G[PPC * c : PPC * (c + 1)] = buf.reshape(PPC, F)

    gids = np.arange(N_CORES * P)          # global partition index
    cls = gids // PPC                      # class of each partition (>=10 pad)
    in_maps = []
    for k in range(N_CORES):
        m = np.zeros((P, MCOLS), dtype=np.float32)
        kcls = cls[P * k : P * (k + 1)]
        for p in range(P):
            if kcls[p] < CLASSES:
                m[p, kcls[p]] = 1.0
        in_maps.append(
            {"x": np.ascontiguousarray(G[P * k : P * (k + 1)]), "mask": m}
        )
    return in_maps


_NC = None


def kernel(X: np.ndarray) -> np.ndarray:
    global _NC
    if _NC is None:
        _NC = _build_bass()
    in_maps = _prep_in_maps(X)
    res = run_bass_kernel_spmd(_NC, in_maps, core_ids=list(range(N_CORES)))
    out = np.zeros(CLASSES, dtype=np.float32)
    for k in range(N_CORES):
        out += res.results[k]["y"].reshape(-1)[:CLASSES]
    return out.astype(np.float32)


# revision 7
# speedup vs baseline: 1.4165x; 1.0276x over previous
"""Bass/Trainium2 kernel for nn_CMOS_60181081752266.

Computes, for each of 10 classes, sum(|patch|) where patch is a static
273x273 crop of the 8192x8192 input X. Only ~3MB of X is ever needed, so
the host slices the 10 patches out of X and repacks them so that class
boundaries fall at *static column positions*, identical on every core
(SPMD-friendly):

  core k input x [128, 729] f32:
    cols [0, 583)   = class k          (74529 elems padded to 128x583)
    cols [583, 729) = quarter (k%4) of class 8 + k//4  (padded 128x584,
                      cols [146q, 146q+146))

Per core:
  - x is DMA'd in 5 column chunks over all three DMA queues (sync HWDGE,
    scalar HWDGE, gpsimd SWDGE) to hide HBM latency
  - DVE reduces each chunk with apply_absolute_value into separate
    columns of pad [128, 32] (chunk partials never mix classes)
  - 4 32x32 block transposes + reduce + one more transpose collapse the
    per-partition partials into 5 scalars in one partition row
  - one 20B DMA (single descriptor -> single DRAM-write receipt) writes
    y [1, 5] = (4 class-A chunk partials, 1 class-B partial)

Host sums the handful of partials per class across cores.
"""

import numpy as np

import concourse.bass as bass
import concourse.mybir as mybir
from concourse.bass_utils import run_bass_kernel_spmd

CLASSES = 10
FRAME_S = 273          # 8192 // (10*3)
N_CORES = 8
P = 128                # SBUF partitions per core
FA = 583               # class-A columns: 128*583 = 74624 >= 273*273
FB = 146               # class-B quarter columns: 4*146 = 584, 128*584 >= 74529
F = FA + FB            # 729
MCOLS = 32
YCOLS = 5
# chunk boundaries: 4 class-A chunks + 1 class-B chunk
CUTS = [0, 150, 300, 450, FA, F]


def _starts():
    # cols = floor(sqrt(10)) + 1 = 4, cell = 8192 // 4 = 2048
    # xs = (i%4)*2048 + 1024 - 136, ys = (i//4)*2048 + 1024 - 136
    return [(888 + 2048 * (i % 4), 888 + 2048 * (i // 4)) for i in range(CLASSES)]


def _build_bass():
    f32 = mybir.dt.float32
    nc = bass.Bass()
    x = nc.dram_tensor("x", [P, F], f32, kind="ExternalInput")
    y = nc.dram_tensor("y", [1, YCOLS], f32, kind="ExternalOutput")
    with (
        nc.sbuf_tensor("t", [P, F], f32) as t,
        nc.sbuf_tensor("pad", [P, MCOLS], f32) as pad,
        nc.sbuf_tensor("tt", [MCOLS, P], f32) as tt,
        nc.sbuf_tensor("ztile", [MCOLS, MCOLS], f32) as ztile,
        nc.sbuf_tensor("zt2", [MCOLS, MCOLS], f32) as zt2,
        nc.semaphore("c0sem") as c0sem,
        nc.semaphore("c1sem") as c1sem,
        nc.semaphore("c2sem") as c2sem,
        nc.semaphore("c3sem") as c3sem,
        nc.semaphore("c4sem") as c4sem,
        nc.semaphore("ysem") as ysem,
        nc.semaphore("gsem") as gsem,
        nc.semaphore("vsem") as vsem,
        nc.Block() as block,
    ):

        @block.gpsimd
        def _(gpsimd):
            gpsimd.memset(pad[:], 0.0)
            gpsimd.memset(ztile[:], 0.0).then_inc(gsem, 1)
            gpsimd.dma_start(
                t[:, CUTS[4] : CUTS[5]], x[:, CUTS[4] : CUTS[5]]
            ).then_inc(c4sem, 16)

        @block.sync
        def _(sync):
            sync.dma_start(t[:, CUTS[0] : CUTS[1]], x[:, CUTS[0] : CUTS[1]]).then_inc(
                c0sem, 16
            )
            sync.dma_start(t[:, CUTS[2] : CUTS[3]], x[:, CUTS[2] : CUTS[3]]).then_inc(
                c2sem, 16
            )
            sync.wait_ge(vsem, 1)
            sync.dma_start(y[:], zt2[0:1, 0:YCOLS]).then_inc(ysem, 16)
            sync.wait_ge(ysem, 16)

        @block.scalar
        def _(scalar):
            scalar.dma_start(
                t[:, CUTS[1] : CUTS[2]], x[:, CUTS[1] : CUTS[2]]
            ).then_inc(c1sem, 16)
            scalar.dma_start(
                t[:, CUTS[3] : CUTS[4]], x[:, CUTS[3] : CUTS[4]]
            ).then_inc(c3sem, 16)

        @block.vector
        def _(vector):
            vector.wait_ge(gsem, 1)
            sems = [c0sem, c1sem, c2sem, c3sem, c4sem]
            # consume chunks roughly in expected completion order
            order = [4, 0, 1, 2, 3]
            for j in order:
                vector.wait_ge(sems[j], 16)
                vector.tensor_reduce(
                    pad[:, j : j + 1],
                    t[:, CUTS[j] : CUTS[j + 1]],
                    axis=mybir.AxisListType.X,
                    op=mybir.AluOpType.add,
                    apply_absolute_value=True,
                )
            vector.drain()
            for b in range(P // MCOLS):
                vector.transpose(
                    tt[0:MCOLS, b * MCOLS : (b + 1) * MCOLS],
                    pad[b * MCOLS : (b + 1) * MCOLS, 0:MCOLS],
                )
            vector.drain()
            vector.tensor_reduce(
                ztile[:, 0:1],
                tt[:],
                axis=mybir.AxisListType.X,
                op=mybir.AluOpType.add,
            )
            vector.drain()
            vector.transpose(zt2[:], ztile[:]).then_inc(vsem, 1)

    return nc


def _prep_in_maps(X: np.ndarray):
    X = np.ascontiguousarray(X, dtype=np.float32)
    starts = _starts()
    flats = []
    for c, (xs, ys) in enumerate(starts):
        flats.append(X[xs : xs + FRAME_S, ys : ys + FRAME_S].reshape(-1))

    in_maps = []
    for k in range(N_CORES):
        xk = np.zeros((P, F), dtype=np.float32)
        # class A = class k
        bufA = np.zeros(P * FA, dtype=np.float32)
        bufA[: FRAME_S * FRAME_S] = flats[k]
        xk[:, :FA] = bufA.reshape(P, FA)
        # class B = quarter (k%4) of class 8 + k//4
        cb = 8 + k // 4
        q = k % 4
        bufB = np.zeros(P * 4 * FB, dtype=np.float32)
        bufB[: FRAME_S * FRAME_S] = flats[cb]
        xk[:, FA:] = bufB.reshape(P, 4 * FB)[:, FB * q : FB * (q + 1)]
        in_maps.append({"x": xk})
    return in_maps


_NC = None


def kernel(X: np.ndarray) -> np.ndarray:
    global _NC
    if _NC is None:
        _NC = _build_bass()
    in_maps = _prep_in_maps(X)
    res = run_bass_kernel_spmd(_NC, in_maps, core_ids=list(range(N_CORES)))
    out = np.zeros(CLASSES, dtype=np.float32)
    for k in range(N_CORES):
        yk = res.results[k]["y"].reshape(-1)
        out[k] += yk[0:4].sum(dtype=np.float32)
        out[8 + k // 4] += yk[4]
    return out.astype(np.float32)
